# revision 8
# baseline (speedup 1.0000x reference)
"""Multi-head attention kernel for Trainium2 (Bass/Tile), 8-core SPMD.

Strategy: data-parallel over batch (B=8 -> one batch element per core).
Each core computes the full attention + fc for its batch element:
  kq = v @ W.T + b  (computed transposed: kqT[j, t]; per head rows 0-63 = k,
                     64-127 = q, q pre-scaled by 1/sqrt(dk) on the host;
                     bias added via a rank-1 matmul into the PSUM group)
  O1: attn[q,k] per head -> softmax sums (no max subtraction; logits are
      tiny) -> normalized attn written to DRAM (the attn output)
  O2: attnT[k,q] per head, with -ln(sum[q]) folded in as a rank-1 matmul
      accumulated into the same PSUM group -> exp gives normalized attnT
  AV: outT[d,q] (lhsT = natural v tiles, rhs = attnT) -> no transposes
  FC: out[q,c] accumulated over heads in SBUF; bias via rank-1 broadcast.

QK/projection matmuls run as float32r (1-pass reduced-precision fp32) to
keep the attn output accurate; AV and FC run in bf16 (2x rate + fast
weight load), which only touches the fc output at the ~0.5% level.

k/q tiles are packed in head pairs (head 2i at partitions 0-63, 2i+1 at
64-127) so matmul lhsT/rhs base partitions always match; the shift from
the projection PSUM layout is done with SBUF->SBUF DMAs.  The phase-A
head loop is software-pipelined (projection runs two heads ahead of the
softmax) so the PE never idles long enough for HAM to re-throttle.
"""

import numpy as np

H = 16
T = 512
D = 1024
DK = 64
B = 8
SCALE = DK ** -0.5

_cached_nc = None

# test.py can flip these; the grading path never touches them.
TRACE = False
TRACE_KW = {}
LAST_RESULTS = None


def _build():
    from contextlib import ExitStack

    import concourse.bacc as bacc
    import concourse.mybir as mybir
    import concourse.tile as tile
    from concourse.masks import make_identity

    f32 = mybir.dt.float32
    f32r = mybir.dt.float32r
    bf16 = mybir.dt.bfloat16
    AF = mybir.ActivationFunctionType

    nc = bacc.Bacc("TRN2", target_bir_lowering=False, debug=False)

    v_d = nc.dram_tensor("v", [T, D], f32r, kind="ExternalInput")
    v16_d = nc.dram_tensor("v16", [T, D], bf16, kind="ExternalInput")
    mneg_d = nc.dram_tensor("mneg", [T, T], f32, kind="ExternalInput")
    mnegT_d = nc.dram_tensor("mnegT", [T, T], f32, kind="ExternalInput")
    wt_d = nc.dram_tensor("wt", [D, 2 * H * DK], f32r, kind="ExternalInput")
    bkq_d = nc.dram_tensor("bkq", [1, 2 * H * DK], f32r, kind="ExternalInput")
    fcwt_d = nc.dram_tensor("fcwt", [H * D, D], bf16, kind="ExternalInput")
    fcb_d = nc.dram_tensor("fcb", [1, D], f32r, kind="ExternalInput")
    out_d = nc.dram_tensor("out", [T, D], f32, kind="ExternalOutput")
    attn_d = nc.dram_tensor("attn", [H, T, T], f32, kind="ExternalOutput")

    vap = v_d.ap()
    outap = out_d.ap()
    attnap = attn_d.ap()

    with tile.TileContext(nc) as tc, ExitStack() as ctx:
        persist = ctx.enter_context(tc.tile_pool(name="persist", bufs=1))
        ps_big = ctx.enter_context(tc.tile_pool(name="psb", bufs=6, space="PSUM"))
        ps_sm = ctx.enter_context(tc.tile_pool(name="pss", bufs=2, space="PSUM"))

        ident_f = persist.tile([128, 128], f32)
        make_identity(nc, ident_f[:])
        ident = persist.tile([128, 128], f32r)
        nc.vector.tensor_copy(ident[:], ident_f[:])

        const_f = persist.tile([1, 1024], f32)
        nc.gpsimd.memset(const_f[:, 0:128], -1.0)
        nc.gpsimd.memset(const_f[:, 128:256], 1.0)
        nc.gpsimd.memset(const_f[:, 512:1024], 1.0)
        negones = persist.tile([1, 128], f32r)
        nc.vector.tensor_copy(negones[:], const_f[:, 0:128])
        ones = persist.tile([1, 128], f32r)
        nc.vector.tensor_copy(ones[:], const_f[:, 128:256])
        ones512 = persist.tile([1, 512], f32r)
        nc.vector.tensor_copy(ones512[:], const_f[:, 512:1024])

        bkq_sb = persist.tile([1, 2 * H * DK], f32r)
        nc.sync.dma_start(bkq_sb[:], bkq_d.ap())
        fcb_sb = persist.tile([1, D], f32r)
        nc.sync.dma_start(fcb_sb[:], fcb_d.ap())

        # fc bias broadcast across partitions via rank-1 matmul
        fcbias = persist.tile([128, D], f32)
        for cn in range(2):
            p = ps_big.tile([128, 512], f32, tag="ps")
            nc.tensor.matmul(
                p[:], ones[:], fcb_sb[:, cn * 512:(cn + 1) * 512],
                start=True, stop=True,
            )
            nc.vector.tensor_copy(fcbias[:, cn * 512:(cn + 1) * 512], p[:])

        # v natural layout (f32r for the vT transposes; bf16 copy in phase B)
        v_sb = persist.tile([128, 4 * D], f32r)
        for tp in range(4):
            nc.sync.dma_start(
                v_sb[:, tp * D:(tp + 1) * D], vap[tp * 128:(tp + 1) * 128, :]
            )

        # head-pair packed k/q: head 2i+par at partitions par*64..par*64+64,
        # pair i at cols i*T
        kT_p = persist.tile([128, 8 * T], f32r)
        qT_p = persist.tile([128, 8 * T], f32r)
        statsT = persist.tile([H, T], f32r)        # row h = ln(sum_h[q])
        fc_acc = persist.tile([128, 4 * D], f32)  # q-tile m at cols m*D

        def kq_sl(base, h, cols):
            i, par = h // 2, h % 2
            p0 = par * 64
            return base[p0:p0 + 64, i * T + cols.start: i * T + cols.stop]

        # ---------------- phase A: kq projection + O1 softmax ----------------
        with ExitStack() as ctxA:
            poolA = ctxA.enter_context(tc.tile_pool(name="phaseA", bufs=1))
            wA = ctxA.enter_context(tc.tile_pool(name="workA", bufs=3))

            sums_all = poolA.tile([128, 4 * H], f32)   # col m*H+h = sum
            stats_ln = poolA.tile([128, 4 * H], f32r)  # ln of the above
            mneg_sb = poolA.tile([128, 4 * T], f32)
            for m in range(4):
                nc.sync.dma_start(
                    mneg_sb[:, m * T:(m + 1) * T],
                    mneg_d.ap()[m * 128:(m + 1) * 128, :],
                )

            wt_sb = poolA.tile([128, 8 * 2048], f32r)  # d-chunk dp at cols dp*2048
            for dp in range(8):
                nc.sync.dma_start(
                    wt_sb[:, dp * 2048:(dp + 1) * 2048],
                    wt_d.ap()[dp * 128:(dp + 1) * 128, :],
                )

            vT = poolA.tile([128, 8 * T], f32r)  # d-chunk dp at cols dp*T
            for dp in range(8):
                for tp in range(4):
                    p = ps_sm.tile([128, 128], f32r, tag="ptr")
                    nc.tensor.transpose(
                        p[:],
                        v_sb[:, tp * D + dp * 128: tp * D + (dp + 1) * 128],
                        ident[:],
                    )
                    nc.vector.tensor_copy(
                        vT[:, dp * T + tp * 128: dp * T + (tp + 1) * 128], p[:]
                    )

            def emit_kqproj(h):
                pkq = ps_big.tile([128, 512], f32, tag="ps")
                for dp in range(8):
                    nc.tensor.matmul(
                        pkq[:],
                        wt_sb[:, dp * 2048 + h * 128: dp * 2048 + (h + 1) * 128],
                        vT[:, dp * T:(dp + 1) * T],
                        start=(dp == 0), stop=False,
                    )
                # bias via rank-1: psum[r, t] += bkq[h*128 + r] * 1
                nc.tensor.matmul(
                    pkq[:], bkq_sb[:, h * 128:(h + 1) * 128], ones512[:],
                    start=False, stop=True,
                )
                kq_stage = wA.tile([128, T], f32r, tag="kqst")
                nc.vector.tensor_copy(kq_stage[:], pkq[:])
                # partition-shift k and q into the packed tiles
                nc.sync.dma_start(kq_sl(kT_p, h, slice(0, T)), kq_stage[0:64, :])
                nc.sync.dma_start(kq_sl(qT_p, h, slice(0, T)), kq_stage[64:128, :])

            def emit_o1(h):
                kT_h = kq_sl(kT_p, h, slice(0, T))
                for m in range(4):
                    po1 = ps_big.tile([128, 512], f32, tag="ps")
                    nc.tensor.matmul(
                        po1[:],
                        kq_sl(qT_p, h, slice(m * 128, (m + 1) * 128)),
                        kT_h,
                        start=True, stop=True,
                    )
                    tmp = wA.tile([128, T], f32, tag="tmp")
                    nc.vector.tensor_add(
                        tmp[:], po1[:], mneg_sb[:, m * T:(m + 1) * T]
                    )
                    e1 = wA.tile([128, T], f32, tag="e1")
                    col = m * H + h
                    nc.scalar.activation(
                        e1[:], tmp[:], AF.Exp,
                        accum_out=sums_all[:, col:col + 1],
                    )
                    rcp = wA.tile([128, 1], f32, tag="rcp")
                    nc.vector.reciprocal(rcp[:], sums_all[:, col:col + 1])
                    ao = wA.tile([128, T], f32, tag="ao")
                    nc.scalar.activation(ao[:], e1[:], AF.Copy, scale=rcp[:])
                    nc.sync.dma_start(attnap[h, m * 128:(m + 1) * 128, :], ao[:])

            # software pipeline: projection runs two heads ahead of softmax
            emit_kqproj(0)
            emit_kqproj(1)
            for h in range(H):
                if h + 2 < H:
                    emit_kqproj(h + 2)
                emit_o1(h)

            # ln of all sums in one ACT op, then transpose -> statsT rows
            nc.scalar.activation(stats_ln[:], sums_all[:], AF.Ln)
            for m in range(4):
                pst = ps_sm.tile([128, 128], f32r, tag="ptr")
                nc.tensor.transpose(
                    pst[:H, :], stats_ln[:, m * H:(m + 1) * H], ident[:]
                )
                nc.vector.tensor_copy(statsT[:, m * 128:(m + 1) * 128], pst[:H, :])

        # ---------------- phase B: O2 + AV + FC ----------------
        with ExitStack() as ctxB:
            poolB = ctxB.enter_context(tc.tile_pool(name="phaseB", bufs=1))
            fcw_pool = ctxB.enter_context(tc.tile_pool(name="fcw", bufs=10))
            wB = ctxB.enter_context(tc.tile_pool(name="workB", bufs=3))
            outT_pool = ctxB.enter_context(tc.tile_pool(name="outT", bufs=2))
            eT_pool = ctxB.enter_context(tc.tile_pool(name="eT", bufs=6))

            v16_sb = poolB.tile([128, 4 * D], bf16)
            for tp in range(4):
                nc.sync.dma_start(
                    v16_sb[:, tp * D:(tp + 1) * D],
                    v16_d.ap()[tp * 128:(tp + 1) * 128, :],
                )
            mnegT_sb = poolB.tile([128, 4 * T], f32)
            for m in range(4):
                nc.sync.dma_start(
                    mnegT_sb[:, m * T:(m + 1) * T],
                    mnegT_d.ap()[m * 128:(m + 1) * 128, :],
                )

            for h in range(H):
                # stats row for head h shifted to partition 0
                st_h = wB.tile([1, T], f32r, tag="sth")
                nc.sync.dma_start(st_h[:], statsT[h:h + 1, :])

                qT_h = kq_sl(qT_p, h, slice(0, T))
                eTs = []
                for kp in range(4):
                    po2 = ps_big.tile([128, 512], f32, tag="ps")
                    nc.tensor.matmul(
                        po2[:],
                        kq_sl(kT_p, h, slice(kp * 128, (kp + 1) * 128)),
                        qT_h,
                        start=True, stop=False,
                    )
                    nc.tensor.matmul(
                        po2[:], negones[:], st_h[:],
                        start=False, stop=True,
                    )
                    tmp2 = wB.tile([128, T], f32, tag="tmp2")
                    nc.vector.tensor_add(
                        tmp2[:], po2[:], mnegT_sb[:, kp * T:(kp + 1) * T]
                    )
                    eTk = eT_pool.tile([128, T], bf16, tag="eT")
                    nc.scalar.activation(eTk[:], tmp2[:], AF.Exp)
                    eTs.append(eTk)

                outT = outT_pool.tile([128, 8 * T], bf16, tag="outT")
                for dp in range(8):
                    pav = ps_big.tile([128, 512], f32, tag="ps")
                    for kp in range(4):
                        nc.tensor.matmul(
                            pav[:],
                            v16_sb[:, kp * D + dp * 128: kp * D + (dp + 1) * 128],
                            eTs[kp][:],
                            start=(kp == 0), stop=(kp == 3),
                        )
                    nc.vector.tensor_copy(outT[:, dp * T:(dp + 1) * T], pav[:])

                fcws = []
                for dp in range(8):
                    fcw = fcw_pool.tile([128, D], bf16, tag="fcw")
                    nc.sync.dma_start(
                        fcw[:],
                        fcwt_d.ap()[h * D + dp * 128: h * D + (dp + 1) * 128, :],
                    )
                    fcws.append(fcw)

                for qm in range(4):
                    for cn in range(2):
                        pfc = ps_big.tile([128, 512], f32, tag="ps")
                        for dp in range(8):
                            nc.tensor.matmul(
                                pfc[:],
                                outT[:, dp * T + qm * 128: dp * T + (qm + 1) * 128],
                                fcws[dp][:, cn * 512:(cn + 1) * 512],
                                start=(dp == 0), stop=(dp == 7),
                            )
                        dst = fc_acc[:, qm * D + cn * 512: qm * D + (cn + 1) * 512]
                        if h == 0:
                            nc.vector.tensor_copy(dst, pfc[:])
                        else:
                            nc.vector.tensor_add(dst, dst, pfc[:])

            for qm in range(4):
                ot = wB.tile([128, D], f32, tag="ot")
                nc.vector.tensor_add(
                    ot[:], fc_acc[:, qm * D:(qm + 1) * D], fcbias[:]
                )
                nc.sync.dma_start(outap[qm * 128:(qm + 1) * 128, :], ot[:])

    nc.compile()
    return nc


def kernel(v, mask, w_kqs_w, w_kqs_b, fc_w, fc_b):
    global _cached_nc, LAST_RESULTS
    import ml_dtypes
    from concourse import bass_utils

    v = np.ascontiguousarray(np.asarray(v, dtype=np.float32))
    mask = np.asarray(mask)
    w_kqs_w = np.asarray(w_kqs_w, dtype=np.float32)
    w_kqs_b = np.asarray(w_kqs_b, dtype=np.float32)
    fc_w = np.asarray(fc_w, dtype=np.float32)
    fc_b = np.asarray(fc_b, dtype=np.float32)

    # host-side weight prep (replicated to all cores)
    scale_col = np.where(
        (np.arange(2 * H * DK) % 128) >= 64, np.float32(SCALE), np.float32(1.0)
    ).astype(np.float32)
    wt = np.ascontiguousarray((w_kqs_w * scale_col[:, None]).T)       # [D, 2HDK]
    bkq = np.ascontiguousarray((w_kqs_b * scale_col).reshape(1, -1))  # [1, 2HDK]
    fcwt = np.ascontiguousarray(fc_w.T.astype(ml_dtypes.bfloat16))    # [H*D, D]
    fcb = np.ascontiguousarray(fc_b.reshape(1, D))
    v16 = v.astype(ml_dtypes.bfloat16)
    mneg = (mask.astype(np.float32) * np.float32(-100000.0))          # [B, T, T]
    mnegT = np.ascontiguousarray(mneg.transpose(0, 2, 1))

    if _cached_nc is None:
        _cached_nc = _build()
    nc = _cached_nc

    in_maps = [
        {
            "v": v[b],
            "v16": v16[b],
            "mneg": mneg[b],
            "mnegT": mnegT[b],
            "wt": wt,
            "bkq": bkq,
            "fcwt": fcwt,
            "fcb": fcb,
        }
        for b in range(B)
    ]
    res = bass_utils.run_bass_kernel_spmd(
        nc, in_maps, core_ids=list(range(B)), trace=TRACE, **TRACE_KW
    )
    LAST_RESULTS = res
    out = np.stack([res.results[b]["out"] for b in range(B)], axis=0)
    attn = np.stack([res.results[b]["attn"] for b in range(B)], axis=1)
    return out, attn
